# revision 1
# baseline (speedup 1.0000x reference)
"""Trainium2 Bass kernel for the weighted/scaled Jensen-Shannon divergence loss.

Math (exactly equivalent to the reference for this data, where the EPS clamps
are never active):
  per valid position with label l and 3-class softmax prob s = sm_l:
    per_pos = 0.5*(s*ln(s) - (1+s)*ln(1+s)) + ln(2)
  loss_b  = SCALE * sum_{pos<j_b}(per_pos) / j_b,   j_b = index of sentinel 3
  out     = mean_b(loss_b)

Sharding: pure data parallel over the batch dim, 64 rows per core x 8 cores.

Device layout per core: tiles [128, F]; partition p<64 holds (b=p, s in
[0,8192)), p>=64 holds (b=p-64, s in [8192,16384)).  The validity mask is
computed with a running-max prefix scan over labels (mask = cummax(lab) < 3);
second-half rows are corrected at reduction time when the sentinel sits in the
first half.
"""

import os
import sys

sys.path.insert(0, "/opt/trn_rl_repo")

import numpy as np

import concourse.bass as bass  # noqa: F401  (import registers AP machinery)
import concourse.tile as tile
from concourse import bacc, mybir
from concourse.bass_utils import run_bass_kernel_spmd

N_CORES = 8
B, C, S = 512, 4, 16384
BC = B // N_CORES          # 64 batch rows per core
H = S // 2                 # 8192, half a row per partition
F = 2048                   # chunk size along the free dim
NCHUNK = H // F

W0 = 0.5
SCALE = -1.0 / ((1.0 - W0) * float(np.log(1.0 - W0)))  # = 2/ln2
LN2 = float(np.log(2.0))

f32 = mybir.dt.float32
bf16 = mybir.dt.bfloat16
i32 = mybir.dt.int32
Alu = mybir.AluOpType
Act = mybir.ActivationFunctionType


def build_program(repeats=1):
    nc = bacc.Bacc(
        "TRN2",
        target_bir_lowering=False,
        debug=False,
        num_devices=N_CORES,
    )
    pred_d = nc.dram_tensor("pred", [BC, C, S], f32, kind="ExternalInput").ap()
    lab_d = nc.dram_tensor("labels", [BC, S], i32, kind="ExternalInput").ap()
    out_d = nc.dram_tensor("out", [1, 1], f32, kind="ExternalOutput").ap()

    with tile.TileContext(nc) as tc:
        for _ in range(repeats):
            _body(tc, out_d, pred_d, lab_d)

    nc.compile()
    return nc


def _body(tc, out_d, pred_d, lab_d):
    nc = tc.nc
    from contextlib import ExitStack

    ctx = ExitStack()
    with ctx:
        # pools: bufs=2 where cross-engine/chunk overlap matters, else 1
        io = ctx.enter_context(tc.tile_pool(name="io", bufs=2))
        ep = ctx.enter_context(tc.tile_pool(name="ep", bufs=2))
        wk2 = ctx.enter_context(tc.tile_pool(name="wk2", bufs=2))
        wk1 = ctx.enter_context(tc.tile_pool(name="wk1", bufs=1))
        sm = ctx.enter_context(tc.tile_pool(name="sm", bufs=2))
        fin = ctx.enter_context(tc.tile_pool(name="fin", bufs=1))
        psum = ctx.enter_context(tc.tile_pool(name="psum", bufs=1, space="PSUM"))

        prev_mrun = None
        prev_rowt = None
        cnt_acc = None

        for ci in range(NCHUNK):
            lo = ci * F

            # ---- loads -------------------------------------------------
            a = []
            for c in range(3):
                t = io.tile([128, F], f32, tag=f"a{c}")
                nc.sync.dma_start(t[0:64, :], pred_d[:, c, lo : lo + F])
                nc.sync.dma_start(t[64:128, :], pred_d[:, c, H + lo : H + lo + F])
                a.append(t)
            lab = io.tile([128, F], i32, tag="lab")
            nc.sync.dma_start(lab[0:64, :], lab_d[:, lo : lo + F])
            nc.sync.dma_start(lab[64:128, :], lab_d[:, H + lo : H + lo + F])

            # ---- ACT: e_c = exp(a_c) ------------------------------------
            e = []
            for c in range(3):
                t = ep.tile([128, F], bf16, tag=f"e{c}")
                nc.scalar.activation(t[:], a[c][:], Act.Exp)
                e.append(t)

            # ---- DVE: label select g_c = (lab==c)*e_c; u = sum_c g_c ----
            lab_bf = wk1.tile([128, F], bf16, tag="labbf")
            nc.vector.tensor_copy(lab_bf[:], lab[:])
            g = []
            for c in range(3):
                t = wk1.tile([128, F], bf16, tag=f"g{c}")
                nc.vector.scalar_tensor_tensor(
                    t[:], lab_bf[:], float(c), e[c][:], Alu.is_equal, Alu.mult
                )
                g.append(t)
            u01 = wk1.tile([128, F], bf16, tag="u01")
            nc.vector.tensor_tensor(u01[:], g[0][:], g[1][:], Alu.add)
            u = wk2.tile([128, F], bf16, tag="u")
            # +1e-30 keeps ln(u) finite at the sentinel position (all g == 0)
            nc.vector.scalar_tensor_tensor(
                u[:], u01[:], 1e-30, g[2][:], Alu.add, Alu.add
            )

            z01 = wk1.tile([128, F], bf16, tag="z01")
            nc.vector.tensor_tensor(z01[:], e[0][:], e[1][:], Alu.add)
            z = wk2.tile([128, F], bf16, tag="z")
            nc.vector.tensor_tensor(z[:], z01[:], e[2][:], Alu.add)

            # ---- ACT: logs; DVE: d = ln u - ln Z ------------------------
            lnu = wk2.tile([128, F], bf16, tag="lnu")
            nc.scalar.activation(lnu[:], u[:], Act.Ln)
            lnz = wk2.tile([128, F], bf16, tag="lnz")
            nc.scalar.activation(lnz[:], z[:], Act.Ln)
            d = wk2.tile([128, F], bf16, tag="d")
            nc.vector.tensor_tensor(d[:], lnu[:], lnz[:], Alu.subtract)

            # ---- mask from running max of labels ------------------------
            mrun = sm.tile([128, F], bf16, tag="mrun")
            init = 0.0 if prev_mrun is None else prev_mrun[:, F - 1 : F]
            nc.vector.tensor_tensor_scan(
                mrun[:], lab[:], lab[:], init, Alu.max, Alu.max
            )
            prev_mrun = mrun

            mask = wk1.tile([128, F], bf16, tag="mask")
            cnt_c = sm.tile([128, 1], f32, tag="cntc")
            nc.vector.tensor_scalar(
                mask[:], mrun[:], 3.0, None, Alu.is_lt, Alu.add, accum_out=cnt_c[:]
            )

            # ---- s, lam, T' = s*d - (1+s)*lam ---------------------------
            s_t = wk2.tile([128, F], bf16, tag="s")
            nc.scalar.activation(s_t[:], d[:], Act.Exp)
            lam = wk2.tile([128, F], bf16, tag="lam")
            nc.scalar.activation(lam[:], s_t[:], Act.Ln, bias=1.0)

            q = wk1.tile([128, F], bf16, tag="q")
            nc.vector.tensor_tensor(q[:], s_t[:], d[:], Alu.mult)
            p = wk1.tile([128, F], bf16, tag="p")
            nc.vector.scalar_tensor_tensor(
                p[:], s_t[:], 1.0, lam[:], Alu.add, Alu.mult
            )
            tp = wk1.tile([128, F], bf16, tag="tp")
            nc.vector.tensor_tensor(tp[:], q[:], p[:], Alu.subtract)

            # ---- masked row-sum, chained across chunks ------------------
            scratch = wk1.tile([128, F], bf16, tag="scr")
            rowt_c = sm.tile([128, 1], f32, tag="rowtc")
            nc.vector.scalar_tensor_tensor(
                scratch[:], tp[:], 0.0, mask[:], Alu.add, Alu.mult,
                accum_out=rowt_c[:],
            )
            if prev_rowt is None:
                prev_rowt = rowt_c
            else:
                nrt_ = sm.tile([128, 1], f32, tag="rowt")
                nc.vector.tensor_tensor(nrt_[:], prev_rowt[:], rowt_c[:], Alu.add)
                prev_rowt = nrt_

            if cnt_acc is None:
                cnt_acc = cnt_c
            else:
                nacc = sm.tile([128, 1], f32, tag="cnta")
                nc.vector.tensor_tensor(nacc[:], cnt_acc[:], cnt_c[:], Alu.add)
                cnt_acc = nacc

        # ================= epilogue (tiny tensors) =======================
        rowt = prev_rowt
        sawmax = prev_mrun[:, F - 1 : F]  # [128,1] final scan state

        # move second-half row stats down to partitions 0..63
        hi_t = fin.tile([64, 1], f32, tag="hi_t")
        nc.sync.dma_start(hi_t[:], rowt[64:128, 0:1])
        hi_c = fin.tile([64, 1], f32, tag="hi_c")
        nc.sync.dma_start(hi_c[:], cnt_acc[64:128, 0:1])

        # corr = 1 if sentinel NOT in first half (else second half invalid)
        corr = fin.tile([64, 1], f32, tag="corr")
        nc.vector.tensor_scalar(corr[:], sawmax[0:64, :], 3.0, None, Alu.is_lt)

        hi_t2 = fin.tile([64, 1], f32, tag="hi_t2")
        nc.vector.tensor_tensor(hi_t2[:], hi_t[:], corr[:], Alu.mult)
        hi_c2 = fin.tile([64, 1], f32, tag="hi_c2")
        nc.vector.tensor_tensor(hi_c2[:], hi_c[:], corr[:], Alu.mult)

        bt = fin.tile([64, 1], f32, tag="bt")
        nc.vector.tensor_tensor(bt[:], rowt[0:64, 0:1], hi_t2[:], Alu.add)
        jb = fin.tile([64, 1], f32, tag="jb")
        nc.vector.tensor_tensor(jb[:], cnt_acc[0:64, 0:1], hi_c2[:], Alu.add)

        # loss_b = 0.5*SCALE*bt/jb + SCALE*ln2   (j >= 1 guaranteed by data)
        rj = fin.tile([64, 1], f32, tag="rj")
        nc.vector.reciprocal(rj[:], jb[:])
        t1 = fin.tile([64, 1], f32, tag="t1")
        nc.vector.tensor_tensor(t1[:], bt[:], rj[:], Alu.mult)
        lossb = fin.tile([64, 1], f32, tag="lossb")
        nc.vector.tensor_scalar(
            lossb[:], t1[:], 0.5 * SCALE, SCALE * LN2, Alu.mult, Alu.add
        )

        ones = fin.tile([64, 1], f32, tag="ones")
        nc.vector.memset(ones[:], 1.0)
        acc = psum.tile([1, 1], f32, tag="acc")
        nc.tensor.matmul(acc[:], ones[:], lossb[:])
        outsb = fin.tile([1, 1], f32, tag="outsb")
        nc.vector.tensor_copy(outsb[:], acc[:])
        nc.sync.dma_start(out_d[:, :], outsb[:])


def build_null_program():
    """Same I/O signature class, minimal work — for dispatch-overhead timing."""
    nc = bacc.Bacc(
        "TRN2", target_bir_lowering=False, debug=False, num_devices=N_CORES
    )
    out_d = nc.dram_tensor("out", [1, 1], f32, kind="ExternalOutput").ap()
    with tile.TileContext(nc) as tc:
        with tc.tile_pool(name="fin", bufs=1) as fin:
            t = fin.tile([1, 1], f32, tag="o")
            nc.vector.memset(t[:], 0.0)
            nc.sync.dma_start(out_d[:, :], t[:])
    nc.compile()
    return nc


_compiled = None


def _get_program():
    global _compiled
    if _compiled is None:
        _compiled = build_program()
    return _compiled


def run(pred, labels, trace=False):
    pred = np.ascontiguousarray(np.asarray(pred, dtype=np.float32))
    labels = np.asarray(labels)
    if labels.dtype != np.int32:
        labels = labels.astype(np.int32)
    labels = np.ascontiguousarray(labels)
    assert pred.shape == (B, C, S), pred.shape
    assert labels.shape == (B, S), labels.shape

    nc = _get_program()
    in_maps = []
    for c in range(N_CORES):
        sl = slice(c * BC, (c + 1) * BC)
        in_maps.append({"pred": pred[sl], "labels": labels[sl]})
    res = run_bass_kernel_spmd(
        nc, in_maps, core_ids=list(range(N_CORES)), trace=trace
    )
    total = sum(float(r["out"][0, 0]) for r in res.results)
    return np.float32(total / B), res


def kernel(pred, labels):
    out, _ = run(pred, labels, trace=False)
    return out

